# revision 33
# baseline (speedup 1.0000x reference)
"""Distributed cross-attention kernel for 8 TRN2 NeuronCores.

Reference computation (L=4096, D_MODEL=1024, D_ATTN=D_MID=128):
    q = x @ Wq + bq ; k = z @ Wk + bk ; v = z @ Wv + bv
    y = softmax(q @ k.T / sqrt(128)) @ v

Sharding: query rows (L_x) split 8 ways; each core holds its x shard and a
replicated copy of z / weights, computes k/v locally, and runs a
flash-attention-style pipeline over 8 z-column tiles of 512.

Performance structure (PE roofline ~44.5us/core at 2.4GHz):
 - Warm-up matmuls on a zeroed tile keep the PE p-state ramp at full
   clock while the first DMAs land (TRN2 drops to 1.2/0.65GHz after any
   idle gap and needs 3us of continuous work to return to 2.4GHz).
 - First z-tile and weights are split into small DMAs across four engine
   queues so the first real matmul can start ~4us in.
 - y is accumulated in natural [i, e] layout (lhsT = p-chunk) with a
   ones-column appended to v, so each y-matmul also accumulates the
   softmax denominator -- no separate rowsum reduction, no final
   transposes; the epilogue is reciprocal + scale + DMA per 128-row
   chunk.
 - Steady-state PE order per tile: [s-mms | kv-mms(jt+2) | y-mms |
   v-transposes(jt+2)] so exp() latency and the DVE bias-adds hide
   under matmul work.  The tail defers v7 so the last exps overlap
   remaining matmuls.

Matmul inputs are bf16 (accumulation fp32 in PSUM); no max-subtraction
in the softmax (s ~ N(0,1) here so exp() is safely bounded).
"""
import math
import sys

import numpy as np

sys.path.insert(0, "/opt/trn_rl_repo")

import ml_dtypes  # noqa: E402

import concourse.mybir as mybir  # noqa: E402
from concourse import bacc  # noqa: E402
from concourse.bass_utils import run_bass_kernel_spmd  # noqa: E402
from concourse.tile import TileContext  # noqa: E402

N_CORES = 8
L = 4096
D_MODEL = 1024
D_ATTN = 128
D_MID = 128
LX = L // N_CORES          # 512 query rows per core
N_MC = D_MODEL // 128      # 8 contraction chunks of 128
N_JT = L // 512            # 8 z-column tiles of 512
INV_SQRT_D = 1.0 / math.sqrt(D_ATTN)
N_WARM = 84                # head warm-up matmuls (boot -> first arrivals)
W_PAIR = 12                # warm burst between arrival-gated chunk pairs
W_K1 = 10                  # warm burst before tile-1 k matmuls

F32 = mybir.dt.float32
BF16 = mybir.dt.bfloat16
BF16_NP = ml_dtypes.bfloat16

# test.py sets these to get tracing / timing out of the same code path
TRACE = False
LAST_RESULT = None


def build():
    nc = bacc.Bacc("TRN2", target_bir_lowering=False)

    # Inputs, pre-packed on host so each DMA is one contiguous read:
    #  xc  [128p, 8c, 512i]       x-shard transposed+chunked (c = d_model chunk)
    #  zr  [8jt, 128p, 8c, 512j]  z transposed+chunked+tiled by j
    #  wk/wv/wq [128p, 8c, 128d]  weights as K-major chunks
    #  identb [128, 128] bf16     PE-transpose identity
    #  bs  [128, 3] f32           bq|bk|bv columns
    xc_e = nc.declare_dram_parameter("xc", [128, N_MC, LX], BF16, isOutput=False)
    zr_e = nc.declare_dram_parameter("zr", [N_JT, 128, N_MC, 512], BF16, isOutput=False)
    wk_e = nc.declare_dram_parameter("wk", [128, N_MC, 128], BF16, isOutput=False)
    wv_e = nc.declare_dram_parameter("wv", [128, N_MC, 128], BF16, isOutput=False)
    wq_e = nc.declare_dram_parameter("wq", [128, N_MC, 128], BF16, isOutput=False)
    identb_e = nc.declare_dram_parameter("identb", [128, 128], BF16, isOutput=False)
    bs_e = nc.declare_dram_parameter("bs", [128, 3], F32, isOutput=False)
    # out [128p, 4c, 128e]: y row i = c*128+p  (host re-interleaves)
    out_e = nc.declare_dram_parameter("out", [128, LX // 128, D_MID], F32, isOutput=True)

    with TileContext(nc) as tc:
        with (
            tc.tile_pool(name="consts", bufs=1) as consts,
            tc.tile_pool(name="z0pool", bufs=6) as z0pool,
            tc.tile_pool(name="zpool", bufs=6) as zpool,
            tc.tile_pool(name="kpool", bufs=3) as kpool,
            tc.tile_pool(name="vpool", bufs=2) as vpool,
            tc.tile_pool(name="vnpool", bufs=2) as vnpool,
            tc.tile_pool(name="ppool", bufs=8) as ppool,
            tc.tile_pool(name="epil", bufs=1) as epil,
            tc.tile_pool(name="ps_mm", bufs=3, space="PSUM") as ps_mm,
            tc.tile_pool(name="ps_st", bufs=2, space="PSUM") as ps_st,
            tc.tile_pool(name="ps_tv", bufs=1, space="PSUM") as ps_tv,
            tc.tile_pool(name="ps_yn", bufs=1, space="PSUM") as ps_yn,
        ):
            # ---- warm-up source tile: gpsimd's preamble drains earliest,
            # so its memset unblocks the PE warm-up soonest.
            wz = consts.tile([128, 64], BF16)
            nc.gpsimd.memset(wz, 0.0)

            # ---- DMAs in consumption order.  The sync queue carries the
            # k/v-path stream (wk, zt0 quarters, wv, zt1 halves, zt2..7) in
            # strict FIFO so arrival matches the PE's chunk order; scalar
            # carries the q path (wq, xc); gpsimd the tiny constants.
            wk_s = consts.tile([128, N_MC, 128], BF16)
            wv_s = consts.tile([128, N_MC, 128], BF16)
            wq_s = consts.tile([128, N_MC, 128], BF16)
            identb = consts.tile([128, 128], BF16)
            bs_s = consts.tile([128, 3], F32)
            zt0 = [z0pool.tile([128, 2, 512], BF16, name=f"zt0_{h}", tag="zt0")
                   for h in range(4)]
            zt1 = [z0pool.tile([128, 4, 512], BF16, name=f"zt1_{h}", tag="zt0")
                   for h in range(2)]

            xc_h = [consts.tile([128, 4, LX], BF16, name=f"xc{h}")
                    for h in range(2)]

            nc.sync.dma_start(out=wk_s, in_=wk_e[:, :, :])
            nc.sync.dma_start(out=zt0[0], in_=zr_e[0, :, 0:2, :])
            nc.sync.dma_start(out=zt0[1], in_=zr_e[0, :, 2:4, :])
            nc.sync.dma_start(out=wv_s, in_=wv_e[:, :, :])
            nc.sync.dma_start(out=zt0[2], in_=zr_e[0, :, 4:6, :])
            nc.sync.dma_start(out=zt0[3], in_=zr_e[0, :, 6:8, :])
            nc.sync.dma_start(out=zt1[0], in_=zr_e[1, :, 0:4, :])
            nc.sync.dma_start(out=zt1[1], in_=zr_e[1, :, 4:8, :])
            nc.sync.dma_start(out=wq_s, in_=wq_e[:, :, :])
            nc.sync.dma_start(out=xc_h[0], in_=xc_e[:, 0:4, :])
            nc.sync.dma_start(out=xc_h[1], in_=xc_e[:, 4:8, :])

            zts = {0: None, 1: None}
            for jt in range(2, N_JT):
                zts[jt] = zpool.tile(
                    [128, N_MC, 512], BF16, name=f"zt{jt}", tag="zt"
                )
                nc.sync.dma_start(out=zts[jt], in_=zr_e[jt, :, :, :])

            nc.gpsimd.dma_start(out=identb, in_=identb_e[:, :])
            nc.gpsimd.dma_start(out=bs_s, in_=bs_e[:, :])

            def z_chunk(jt, c):
                if jt == 0:
                    return zt0[c // 2][:, c % 2, :]
                if jt == 1:
                    return zt1[c // 4][:, c % 4, :]
                return zts[jt][:, c, :]

            # ---- PE warm-up: tiny no-dependency matmuls hold full clock
            # while DMAs land; bursts are also woven between arrival-gated
            # matmuls below so idle gaps never reset the p-state ramp.
            # (Borrows the ps_tv pool's bank; freed by program order
            # before the first transpose needs it.)
            warm = ps_tv.tile([64, 64], F32, name="warm", tag="ps_t")

            def warm_burst(n):
                for _ in range(n):
                    nc.tensor.matmul(warm, wz, wz, start=True, stop=True)

            warm_burst(N_WARM)

            # persistent y accumulator [128i, 4c, 256] f32 (2 PSUM banks);
            # col 128 of each c-chunk is the softmax denominator.
            yn = ps_yn.tile([128, LX // 128, 256], F32, name="yn", tag="yn")

            kts = {}
            vns = {}

            def kv_mms(jt, which, cs):
                """Emit k- or v-chunk matmuls for z tile jt."""
                ps = kv_ps[jt][0 if which == "k" else 1]
                w_s = wk_s if which == "k" else wv_s
                for c in cs:
                    nc.tensor.matmul(
                        ps, w_s[:, c, :], z_chunk(jt, c),
                        start=(c == 0), stop=(c == N_MC - 1),
                    )

            kv_ps = {}

            def kv_alloc(jt):
                kv_ps[jt] = (
                    ps_mm.tile([128, 512], F32, name=f"ps_k{jt}", tag="mm"),
                    ps_mm.tile([128, 512], F32, name=f"ps_v{jt}", tag="mm"),
                )

            def kt_post(jt):
                kt = kpool.tile([128, 512], BF16, name=f"kt{jt}", tag="kt")
                nc.vector.tensor_scalar_add(kt, kv_ps[jt][0], bs_s[:, 1:2])
                kts[jt] = kt

            def vt_post(jt):
                vt = vpool.tile([128, 512], BF16, name=f"vt{jt}", tag="vt")
                nc.vector.tensor_scalar_add(vt, kv_ps[jt][1], bs_s[:, 2:3])
                vts[jt] = vt

            vts = {}

            def vn_trans(jt, copy_eng):
                """PE transposes vt -> v-natural chunks + ones column."""
                vn = vnpool.tile([128, 4, 129], BF16, name=f"vn{jt}", tag="vn")
                nc.vector.memset(vn[:, :, 128:129], 1.0)
                pst = ps_tv.tile([128, 4, 128], BF16, name=f"ps_tv{jt}", tag="ps_t")
                for s4 in range(4):
                    nc.tensor.transpose(
                        pst[:, s4, :], vts[jt][:, s4 * 128:(s4 + 1) * 128], identb
                    )
                copy_eng.activation(
                    vn[:, :, 0:128], pst, mybir.ActivationFunctionType.Identity,
                    bias=0.0,
                ) if copy_eng is nc.scalar else copy_eng.tensor_copy(
                    vn[:, :, 0:128], pst
                )
                vns[jt] = vn

            # ---- prologue: kv0/kv1 chunk-paced by the arrival stream, warm
            # bursts absorbing the gaps; then q (xc arrives after zt1).
            kv_alloc(0)
            kv_mms(0, "k", (0, 1))
            warm_burst(W_PAIR)
            kv_mms(0, "k", (2, 3))
            warm_burst(W_PAIR)
            kv_mms(0, "v", (0, 1))
            kv_mms(0, "v", (2, 3))
            warm_burst(W_PAIR)
            kv_mms(0, "k", (4, 5))
            kv_mms(0, "v", (4, 5))
            warm_burst(W_PAIR)
            kv_mms(0, "k", (6, 7))
            kv_mms(0, "v", (6, 7))
            kt_post(0)
            vt_post(0)

            kv_alloc(1)
            warm_burst(W_K1)
            kv_mms(1, "k", (0, 1, 2, 3))
            kv_mms(1, "v", (0, 1, 2, 3))
            warm_burst(W_PAIR)
            kv_mms(1, "k", (4, 5, 6, 7))
            kv_mms(1, "v", (4, 5, 6, 7))
            kt_post(1)
            vt_post(1)

            ps_q = ps_mm.tile([128, LX], F32, name="ps_q", tag="mm")
            for c in range(N_MC):
                nc.tensor.matmul(
                    ps_q, wq_s[:, c, :], xc_h[c // 4][:, c % 4, :],
                    start=(c == 0), stop=(c == N_MC - 1),
                )
            qT_s = consts.tile([128, LX], BF16)
            nc.scalar.activation(
                qT_s, ps_q, mybir.ActivationFunctionType.Identity, bias=bs_s[:, 0:1]
            )

            # per-chunk epilogue tiles: separate tiles so the cross-engine
            # reciprocal/scale/DMA chains carry no false (tile-granular)
            # dependencies between chunks
            y_outs = [epil.tile([128, D_MID], F32, name=f"y_out{c}")
                      for c in range(LX // 128)]
            rsrs = [epil.tile([128, 1], F32, name=f"rsr{c}")
                    for c in range(LX // 128)]
            pts = {}

            def s_mms(jt, s4s):
                """s-matmuls + exp for the given s4 chunks of tile jt."""
                for s4 in s4s:
                    pss = ps_st.tile([128, LX], F32, name=f"ps_s{jt}_{s4}", tag="st")
                    nc.tensor.matmul(
                        pss, kts[jt][:, s4 * 128:(s4 + 1) * 128], qT_s,
                        start=True, stop=True,
                    )
                    pt = ppool.tile([128, LX], BF16, name=f"pt{jt}_{s4}", tag="pt")
                    nc.scalar.activation(
                        pt, pss, mybir.ActivationFunctionType.Exp, scale=INV_SQRT_D
                    )
                    pts[(jt, s4)] = pt

            def y_half(jt, s4s):
                # PSUM `start` zeroes the WHOLE bank, so only the first
                # region per bank (c4 0 and 2) may use it; their bank-wipe
                # doubles as the zero-init for the sibling regions (c4 1, 3)
                # which accumulate with start=False from the first matmul.
                for s4 in s4s:
                    pt = pts[(jt, s4)]
                    for c4 in range(LX // 128):
                        nc.tensor.matmul(
                            yn[:, c4, 0:129],
                            pt[:, c4 * 128:(c4 + 1) * 128], vns[jt][:, s4, :],
                            start=(jt == 0 and s4 == 0 and c4 % 2 == 0),
                            stop=(jt == N_JT - 1 and s4 == 3),
                        )

            # ---- main loop, prefetch-1: iteration jt runs attention on
            # tile jt while computing k/v for tile jt+1, ordered so exp()
            # latency and the DVE adds hide under matmuls:
            #   [s01 | tr(jt) | k(jt+1) c0-3 | s23 | k c4-7 | y01 | v(jt+1) | y23]
            # iter0 emits no kv (kv0/kv1 ran in the prologue) so the z
            # stream gets catch-up slack; iters 1-6 compute kv(jt+1).
            for jt in range(N_JT):
                nxt = jt + 1
                s_mms(jt, (0, 1))
                vn_trans(jt, nc.scalar if nxt < N_JT else nc.vector)
                if jt == 0:
                    s_mms(jt, (2, 3))
                    y_half(jt, (0, 1))
                    y_half(jt, (2, 3))
                elif nxt < N_JT:
                    kv_alloc(nxt)
                    kv_mms(nxt, "k", range(0, 4))
                    s_mms(jt, (2, 3))
                    kv_mms(nxt, "k", range(4, 8))
                    kt_post(nxt)
                    y_half(jt, (0, 1))
                    kv_mms(nxt, "v", range(N_MC))
                    vt_post(nxt)
                    y_half(jt, (2, 3))
                else:
                    # tail: finish all y-matmuls, then per-chunk epilogue
                    # chains split across DVE and Scalar.
                    s_mms(jt, (2, 3))
                    y_half(jt, (0, 1))
                    y_half(jt, (2, 3))
                    dma_engs = [nc.sync, nc.scalar, nc.gpsimd, nc.sync]
                    for c4 in range(LX // 128):
                        nc.vector.reciprocal(rsrs[c4], yn[:, c4, 128:129])
                        if c4 % 2 == 0:
                            nc.vector.tensor_scalar_mul(
                                y_outs[c4], yn[:, c4, 0:128], rsrs[c4]
                            )
                        else:
                            nc.scalar.activation(
                                y_outs[c4], yn[:, c4, 0:128],
                                mybir.ActivationFunctionType.Identity,
                                scale=rsrs[c4],
                            )
                        dma_engs[c4].dma_start(
                            out=out_e[:, c4, :], in_=y_outs[c4]
                        )

    nc.compile()
    return nc


def _pack_kxm(w):
    """[D_MODEL, d] -> [128p, 8c, d] bf16 with m = c*128 + p."""
    d = w.shape[1]
    return np.ascontiguousarray(
        w.reshape(N_MC, 128, d).transpose(1, 0, 2).astype(BF16_NP)
    )


def kernel(x, z, Wq, bq, Wk, bk, Wv, bv):
    global LAST_RESULT
    x = np.asarray(x, dtype=np.float32)
    z = np.asarray(z, dtype=np.float32)

    zT = np.ascontiguousarray(z.T)                      # [1024, 4096]
    # [8c, 128p, 8jt, 512j] -> [jt, p, c, j]
    zr = np.ascontiguousarray(
        zT.reshape(N_MC, 128, N_JT, 512).transpose(2, 1, 0, 3).astype(BF16_NP)
    )
    xT = np.ascontiguousarray(x.T)                      # [1024, 4096]
    bs = np.ascontiguousarray(
        np.stack(
            [
                np.asarray(bq, dtype=np.float32),
                np.asarray(bk, dtype=np.float32),
                np.asarray(bv, dtype=np.float32),
            ],
            axis=1,
        )
    )
    wk = _pack_kxm(np.asarray(Wk, dtype=np.float32))
    wv = _pack_kxm(np.asarray(Wv, dtype=np.float32))
    wq = _pack_kxm(np.asarray(Wq, dtype=np.float32))
    identb = np.eye(128, dtype=BF16_NP)

    in_maps = []
    for cid in range(N_CORES):
        xs = xT[:, cid * LX:(cid + 1) * LX]             # [1024, 512]
        xc = np.ascontiguousarray(
            xs.reshape(N_MC, 128, LX).transpose(1, 0, 2).astype(BF16_NP)
        )
        in_maps.append(
            {"xc": xc, "zr": zr, "wk": wk, "wv": wv, "wq": wq,
             "identb": identb, "bs": bs}
        )

    nc = build()
    res = run_bass_kernel_spmd(
        nc, in_maps, core_ids=list(range(N_CORES)), trace=TRACE
    )
    LAST_RESULT = res

    out = np.empty((L, D_MID), dtype=np.float32)
    for cid in range(N_CORES):
        o = res.results[cid]["out"]                     # [128, 4, 128]
        out[cid * LX:(cid + 1) * LX] = np.asarray(o).transpose(1, 0, 2).reshape(LX, D_MID)
    return out


# revision 34
# speedup vs baseline: 1.1566x; 1.1566x over previous
"""Distributed cross-attention kernel for 8 TRN2 NeuronCores.

Reference computation (L=4096, D_MODEL=1024, D_ATTN=D_MID=128):
    q = x @ Wq + bq ; k = z @ Wk + bk ; v = z @ Wv + bv
    y = softmax(q @ k.T / sqrt(128)) @ v

Sharding: query rows (L_x) split 8 ways; each core holds its x shard and a
replicated copy of z / weights, computes k/v locally, and runs a
flash-attention-style pipeline over 8 z-column tiles of 512.

Performance structure (PE roofline ~44.5us/core at 2.4GHz):
 - Warm-up matmuls on a zeroed tile keep the PE p-state ramp at full
   clock while the first DMAs land (TRN2 drops to 1.2/0.65GHz after any
   idle gap and needs 3us of continuous work to return to 2.4GHz).
 - First z-tile and weights are split into small DMAs across four engine
   queues so the first real matmul can start ~4us in.
 - y is accumulated in natural [i, e] layout (lhsT = p-chunk) with a
   ones-column appended to v, so each y-matmul also accumulates the
   softmax denominator -- no separate rowsum reduction, no final
   transposes; the epilogue is reciprocal + scale + DMA per 128-row
   chunk.
 - Steady-state PE order per tile: [s-mms | kv-mms(jt+2) | y-mms |
   v-transposes(jt+2)] so exp() latency and the DVE bias-adds hide
   under matmul work.  The tail defers v7 so the last exps overlap
   remaining matmuls.

Matmul inputs are bf16 (accumulation fp32 in PSUM); no max-subtraction
in the softmax (s ~ N(0,1) here so exp() is safely bounded).
"""
import math
import sys

import numpy as np

sys.path.insert(0, "/opt/trn_rl_repo")

import ml_dtypes  # noqa: E402

import concourse.mybir as mybir  # noqa: E402
from concourse import bacc  # noqa: E402
from concourse.bass_utils import run_bass_kernel_spmd  # noqa: E402
from concourse.tile import TileContext  # noqa: E402

N_CORES = 8
L = 4096
D_MODEL = 1024
D_ATTN = 128
D_MID = 128
LX = L // N_CORES          # 512 query rows per core
N_MC = D_MODEL // 128      # 8 contraction chunks of 128
N_JT = L // 512            # 8 z-column tiles of 512
INV_SQRT_D = 1.0 / math.sqrt(D_ATTN)
N_WARM = 44                # head warm-up matmuls (boot -> first arrivals)
W_PAIR = 4                 # warm burst between arrival-gated chunk pairs
W_K1 = 6                   # warm burst before tile-1 k matmuls

F32 = mybir.dt.float32
BF16 = mybir.dt.bfloat16
BF16_NP = ml_dtypes.bfloat16

# test.py sets these to get tracing / timing out of the same code path
TRACE = False
LAST_RESULT = None


def build():
    nc = bacc.Bacc("TRN2", target_bir_lowering=False)

    # Inputs, pre-packed on host so each DMA is one contiguous read:
    #  xc  [128p, 8c, 512i]       x-shard transposed+chunked (c = d_model chunk)
    #  zr  [8jt, 128p, 8c, 512j]  z transposed+chunked+tiled by j
    #  wk/wv/wq [128p, 8c, 128d]  weights as K-major chunks
    #  identb [128, 128] bf16     PE-transpose identity
    #  bs  [128, 3] f32           bq|bk|bv columns
    xc_e = nc.declare_dram_parameter("xc", [128, N_MC, LX], BF16, isOutput=False)
    zr_e = nc.declare_dram_parameter("zr", [N_JT, 128, N_MC, 512], BF16, isOutput=False)
    wk_e = nc.declare_dram_parameter("wk", [128, N_MC, 128], BF16, isOutput=False)
    wv_e = nc.declare_dram_parameter("wv", [128, N_MC, 128], BF16, isOutput=False)
    wq_e = nc.declare_dram_parameter("wq", [128, N_MC, 128], BF16, isOutput=False)
    identb_e = nc.declare_dram_parameter("identb", [128, 128], BF16, isOutput=False)
    bs_e = nc.declare_dram_parameter("bs", [128, 3], F32, isOutput=False)
    # out [128p, 4c, 128e]: y row i = c*128+p  (host re-interleaves)
    out_e = nc.declare_dram_parameter("out", [128, LX // 128, D_MID], F32, isOutput=True)

    with TileContext(nc) as tc:
        with (
            tc.tile_pool(name="consts", bufs=1) as consts,
            tc.tile_pool(name="z0pool", bufs=6) as z0pool,
            tc.tile_pool(name="zpool", bufs=6) as zpool,
            tc.tile_pool(name="kpool", bufs=3) as kpool,
            tc.tile_pool(name="vpool", bufs=2) as vpool,
            tc.tile_pool(name="vnpool", bufs=2) as vnpool,
            tc.tile_pool(name="ppool", bufs=8) as ppool,
            tc.tile_pool(name="epil", bufs=1) as epil,
            tc.tile_pool(name="ps_mm", bufs=3, space="PSUM") as ps_mm,
            tc.tile_pool(name="ps_st", bufs=2, space="PSUM") as ps_st,
            tc.tile_pool(name="ps_tv", bufs=1, space="PSUM") as ps_tv,
            tc.tile_pool(name="ps_yn", bufs=1, space="PSUM") as ps_yn,
        ):
            # ---- warm-up source tile: gpsimd's preamble drains earliest,
            # so its memset unblocks the PE warm-up soonest.
            wz = consts.tile([128, 64], BF16)
            nc.gpsimd.memset(wz, 0.0)

            # ---- DMAs in consumption order.  The sync queue carries the
            # k/v-path stream (wk, zt0 quarters, wv, zt1 halves, zt2..7) in
            # strict FIFO so arrival matches the PE's chunk order; scalar
            # carries the q path (wq, xc); gpsimd the tiny constants.
            wk_s = consts.tile([128, N_MC, 128], BF16)
            wv_s = consts.tile([128, N_MC, 128], BF16)
            wq_s = consts.tile([128, N_MC, 128], BF16)
            identb = consts.tile([128, 128], BF16)
            bs_s = consts.tile([128, 3], F32)
            zt0 = [z0pool.tile([128, 2, 512], BF16, name=f"zt0_{h}", tag="zt0")
                   for h in range(4)]
            zt1 = [z0pool.tile([128, 4, 512], BF16, name=f"zt1_{h}", tag="zt0")
                   for h in range(2)]

            xc_h = [consts.tile([128, 4, LX], BF16, name=f"xc{h}")
                    for h in range(2)]

            nc.sync.dma_start(out=wk_s, in_=wk_e[:, :, :])
            nc.sync.dma_start(out=zt0[0], in_=zr_e[0, :, 0:2, :])
            nc.sync.dma_start(out=zt0[1], in_=zr_e[0, :, 2:4, :])
            nc.sync.dma_start(out=wv_s, in_=wv_e[:, :, :])
            nc.sync.dma_start(out=zt0[2], in_=zr_e[0, :, 4:6, :])
            nc.sync.dma_start(out=zt0[3], in_=zr_e[0, :, 6:8, :])
            nc.sync.dma_start(out=zt1[0], in_=zr_e[1, :, 0:4, :])
            nc.sync.dma_start(out=zt1[1], in_=zr_e[1, :, 4:8, :])
            nc.sync.dma_start(out=wq_s, in_=wq_e[:, :, :])
            nc.sync.dma_start(out=xc_h[0], in_=xc_e[:, 0:4, :])
            nc.sync.dma_start(out=xc_h[1], in_=xc_e[:, 4:8, :])

            zts = {0: None, 1: None}
            for jt in range(2, N_JT):
                zts[jt] = zpool.tile(
                    [128, N_MC, 512], BF16, name=f"zt{jt}", tag="zt"
                )
                nc.sync.dma_start(out=zts[jt], in_=zr_e[jt, :, :, :])

            nc.gpsimd.dma_start(out=identb, in_=identb_e[:, :])
            nc.gpsimd.dma_start(out=bs_s, in_=bs_e[:, :])

            def z_chunk(jt, c):
                if jt == 0:
                    return zt0[c // 2][:, c % 2, :]
                if jt == 1:
                    return zt1[c // 4][:, c % 4, :]
                return zts[jt][:, c, :]

            # ---- PE warm-up: tiny no-dependency matmuls hold full clock
            # while DMAs land; bursts are also woven between arrival-gated
            # matmuls below so idle gaps never reset the p-state ramp.
            # (Borrows the ps_tv pool's bank; freed by program order
            # before the first transpose needs it.)
            warm = ps_tv.tile([64, 64], F32, name="warm", tag="ps_t")

            def warm_burst(n):
                for _ in range(n):
                    nc.tensor.matmul(warm, wz, wz, start=True, stop=True)

            warm_burst(N_WARM)

            # persistent y accumulator [128i, 4c, 256] f32 (2 PSUM banks);
            # col 128 of each c-chunk is the softmax denominator.
            yn = ps_yn.tile([128, LX // 128, 256], F32, name="yn", tag="yn")

            kts = {}
            vns = {}

            def kv_mms(jt, which, cs):
                """Emit k- or v-chunk matmuls for z tile jt."""
                ps = kv_ps[jt][0 if which == "k" else 1]
                w_s = wk_s if which == "k" else wv_s
                for c in cs:
                    nc.tensor.matmul(
                        ps, w_s[:, c, :], z_chunk(jt, c),
                        start=(c == 0), stop=(c == N_MC - 1),
                    )

            kv_ps = {}

            def kv_alloc(jt):
                kv_ps[jt] = (
                    ps_mm.tile([128, 512], F32, name=f"ps_k{jt}", tag="mm"),
                    ps_mm.tile([128, 512], F32, name=f"ps_v{jt}", tag="mm"),
                )

            def kt_post(jt):
                kt = kpool.tile([128, 512], BF16, name=f"kt{jt}", tag="kt")
                nc.vector.tensor_scalar_add(kt, kv_ps[jt][0], bs_s[:, 1:2])
                kts[jt] = kt

            def vt_post(jt):
                vt = vpool.tile([128, 512], BF16, name=f"vt{jt}", tag="vt")
                nc.vector.tensor_scalar_add(vt, kv_ps[jt][1], bs_s[:, 2:3])
                vts[jt] = vt

            vts = {}

            def vn_trans(jt, copy_eng):
                """PE transposes vt -> v-natural chunks + ones column."""
                vn = vnpool.tile([128, 4, 129], BF16, name=f"vn{jt}", tag="vn")
                nc.vector.memset(vn[:, :, 128:129], 1.0)
                pst = ps_tv.tile([128, 4, 128], BF16, name=f"ps_tv{jt}", tag="ps_t")
                for s4 in range(4):
                    nc.tensor.transpose(
                        pst[:, s4, :], vts[jt][:, s4 * 128:(s4 + 1) * 128], identb
                    )
                copy_eng.activation(
                    vn[:, :, 0:128], pst, mybir.ActivationFunctionType.Identity,
                    bias=0.0,
                ) if copy_eng is nc.scalar else copy_eng.tensor_copy(
                    vn[:, :, 0:128], pst
                )
                vns[jt] = vn

            # ---- prologue: kv0/kv1 chunk-paced by the arrival stream, warm
            # bursts absorbing the gaps; then q (xc arrives after zt1).
            kv_alloc(0)
            kv_mms(0, "k", (0, 1))
            warm_burst(W_PAIR)
            kv_mms(0, "k", (2, 3))
            warm_burst(W_PAIR)
            kv_mms(0, "v", (0, 1))
            kv_mms(0, "v", (2, 3))
            warm_burst(W_PAIR)
            kv_mms(0, "k", (4, 5))
            kv_mms(0, "v", (4, 5))
            warm_burst(W_PAIR)
            kv_mms(0, "k", (6, 7))
            kv_mms(0, "v", (6, 7))
            kt_post(0)
            vt_post(0)

            kv_alloc(1)
            warm_burst(W_K1)
            kv_mms(1, "k", (0, 1, 2, 3))
            kv_mms(1, "v", (0, 1, 2, 3))
            warm_burst(W_PAIR)
            kv_mms(1, "k", (4, 5, 6, 7))
            kv_mms(1, "v", (4, 5, 6, 7))
            kt_post(1)
            vt_post(1)

            ps_q = ps_mm.tile([128, LX], F32, name="ps_q", tag="mm")
            for c in range(N_MC):
                nc.tensor.matmul(
                    ps_q, wq_s[:, c, :], xc_h[c // 4][:, c % 4, :],
                    start=(c == 0), stop=(c == N_MC - 1),
                )
            qT_s = consts.tile([128, LX], BF16)
            nc.scalar.activation(
                qT_s, ps_q, mybir.ActivationFunctionType.Identity, bias=bs_s[:, 0:1]
            )

            # per-chunk epilogue tiles: separate tiles so the cross-engine
            # reciprocal/scale/DMA chains carry no false (tile-granular)
            # dependencies between chunks
            y_outs = [epil.tile([128, D_MID], F32, name=f"y_out{c}")
                      for c in range(LX // 128)]
            rsrs = [epil.tile([128, 1], F32, name=f"rsr{c}")
                    for c in range(LX // 128)]
            pts = {}

            def s_mms(jt, s4s):
                """s-matmuls + exp for the given s4 chunks of tile jt."""
                for s4 in s4s:
                    pss = ps_st.tile([128, LX], F32, name=f"ps_s{jt}_{s4}", tag="st")
                    nc.tensor.matmul(
                        pss, kts[jt][:, s4 * 128:(s4 + 1) * 128], qT_s,
                        start=True, stop=True,
                    )
                    pt = ppool.tile([128, LX], BF16, name=f"pt{jt}_{s4}", tag="pt")
                    nc.scalar.activation(
                        pt, pss, mybir.ActivationFunctionType.Exp, scale=INV_SQRT_D
                    )
                    pts[(jt, s4)] = pt

            def y_half(jt, s4s):
                # PSUM `start` zeroes the WHOLE bank, so only the first
                # region per bank (c4 0 and 2) may use it; their bank-wipe
                # doubles as the zero-init for the sibling regions (c4 1, 3)
                # which accumulate with start=False from the first matmul.
                for s4 in s4s:
                    pt = pts[(jt, s4)]
                    for c4 in range(LX // 128):
                        nc.tensor.matmul(
                            yn[:, c4, 0:129],
                            pt[:, c4 * 128:(c4 + 1) * 128], vns[jt][:, s4, :],
                            start=(jt == 0 and s4 == 0 and c4 % 2 == 0),
                            stop=(jt == N_JT - 1 and s4 == 3),
                        )

            # ---- main loop, prefetch-1: iteration jt runs attention on
            # tile jt while computing k/v for tile jt+1, ordered so exp()
            # latency and the DVE adds hide under matmuls:
            #   [s01 | tr(jt) | k(jt+1) c0-3 | s23 | k c4-7 | y01 | v(jt+1) | y23]
            # iter0 emits no kv (kv0/kv1 ran in the prologue) so the z
            # stream gets catch-up slack; iters 1-6 compute kv(jt+1).
            for jt in range(N_JT):
                nxt = jt + 1
                s_mms(jt, (0, 1))
                vn_trans(jt, nc.scalar if nxt < N_JT else nc.vector)
                if jt == 0:
                    s_mms(jt, (2, 3))
                    y_half(jt, (0, 1))
                    y_half(jt, (2, 3))
                elif nxt < N_JT:
                    kv_alloc(nxt)
                    kv_mms(nxt, "k", range(0, 4))
                    s_mms(jt, (2, 3))
                    kv_mms(nxt, "k", range(4, 8))
                    kt_post(nxt)
                    y_half(jt, (0, 1))
                    kv_mms(nxt, "v", range(N_MC))
                    vt_post(nxt)
                    y_half(jt, (2, 3))
                else:
                    # tail: finish all y-matmuls, then per-chunk epilogue
                    # chains split across DVE and Scalar.
                    s_mms(jt, (2, 3))
                    y_half(jt, (0, 1))
                    y_half(jt, (2, 3))
                    dma_engs = [nc.sync, nc.scalar, nc.gpsimd, nc.sync]
                    for c4 in range(LX // 128):
                        nc.vector.reciprocal(rsrs[c4], yn[:, c4, 128:129])
                        if c4 % 2 == 0:
                            nc.vector.tensor_scalar_mul(
                                y_outs[c4], yn[:, c4, 0:128], rsrs[c4]
                            )
                        else:
                            nc.scalar.activation(
                                y_outs[c4], yn[:, c4, 0:128],
                                mybir.ActivationFunctionType.Identity,
                                scale=rsrs[c4],
                            )
                        dma_engs[c4].dma_start(
                            out=out_e[:, c4, :], in_=y_outs[c4]
                        )

    nc.compile()
    return nc


def _pack_kxm(w):
    """[D_MODEL, d] -> [128p, 8c, d] bf16 with m = c*128 + p."""
    d = w.shape[1]
    return np.ascontiguousarray(
        w.reshape(N_MC, 128, d).transpose(1, 0, 2).astype(BF16_NP)
    )


def kernel(x, z, Wq, bq, Wk, bk, Wv, bv):
    global LAST_RESULT
    x = np.asarray(x, dtype=np.float32)
    z = np.asarray(z, dtype=np.float32)

    zT = np.ascontiguousarray(z.T)                      # [1024, 4096]
    # [8c, 128p, 8jt, 512j] -> [jt, p, c, j]
    zr = np.ascontiguousarray(
        zT.reshape(N_MC, 128, N_JT, 512).transpose(2, 1, 0, 3).astype(BF16_NP)
    )
    xT = np.ascontiguousarray(x.T)                      # [1024, 4096]
    bs = np.ascontiguousarray(
        np.stack(
            [
                np.asarray(bq, dtype=np.float32),
                np.asarray(bk, dtype=np.float32),
                np.asarray(bv, dtype=np.float32),
            ],
            axis=1,
        )
    )
    wk = _pack_kxm(np.asarray(Wk, dtype=np.float32))
    wv = _pack_kxm(np.asarray(Wv, dtype=np.float32))
    wq = _pack_kxm(np.asarray(Wq, dtype=np.float32))
    identb = np.eye(128, dtype=BF16_NP)

    in_maps = []
    for cid in range(N_CORES):
        xs = xT[:, cid * LX:(cid + 1) * LX]             # [1024, 512]
        xc = np.ascontiguousarray(
            xs.reshape(N_MC, 128, LX).transpose(1, 0, 2).astype(BF16_NP)
        )
        in_maps.append(
            {"xc": xc, "zr": zr, "wk": wk, "wv": wv, "wq": wq,
             "identb": identb, "bs": bs}
        )

    nc = build()
    res = run_bass_kernel_spmd(
        nc, in_maps, core_ids=list(range(N_CORES)), trace=TRACE
    )
    LAST_RESULT = res

    out = np.empty((L, D_MID), dtype=np.float32)
    for cid in range(N_CORES):
        o = res.results[cid]["out"]                     # [128, 4, 128]
        out[cid * LX:(cid + 1) * LX] = np.asarray(o).transpose(1, 0, 2).reshape(LX, D_MID)
    return out
